# revision 11
# baseline (speedup 1.0000x reference)
"""Trainium2 Bass kernel for nn_BoundaryDistillationLoss.

loss = mean((|grad(softmax(s))| - |grad(softmax(t))|)^2) with depthwise 3x3
Sobel gradients, expanded as [ sum(qs) + sum(qt) - 2*sum(sqrt(qs*qt)) ] / N
with q = gx^2 + gy^2.

v6 layout: host pre-casts inputs to bf16 and rearranges each core's shard to
(h, c, w) so every DMA is contiguous.  2048 rows data-parallel over 8 cores;
per core two 128-row slabs (126 output rows each) plus a packed 6-row tail.
On-chip: h-rows on partitions, (c, w) on the free dim, per-4-channel chunk
tiles in a ring so slab N+1's DMA/exp/z-sum/normalize overlaps slab N's conv.
Sobel row-taps are banded 128x128 matmuls; col-taps are +-1-shifted rhs views
of a zero-padded slab.  Per (channel, w-half) all four conv outputs go to one
4-bank PSUM group [gxs|gxt|gys|gyt]: ScalarE squares the gx half while
VectorE runs a fused square-add (SQADD) on the gy half in different banks,
GPSIMD multiplies qs*qt, and one big in-place SQRT per slab (with free
accumulate) produces the cross term without exp<->sqrt table thrashing.
"""

import numpy as np
from contextlib import ExitStack

import concourse.bass as bass
import concourse.bacc as bacc
import concourse.mybir as mybir
import concourse.tile as tile
from concourse import bass_utils
import concourse.dve_ops as dve_ops
from concourse.dve_spec import C0 as _C0, Spec as _Spec, Src0 as _Src0, \
    Src1 as _Src1, lower as _dve_lower, sq as _dve_sq
from concourse.dve_uop import DveOpSpec as _DveOpSpec
from operator import add as _op_add


def _register_custom(name, body, reference):
    for o in dve_ops.OPS:
        if o.name == name:
            return o
    spec = _Spec(body=body, accum=_op_add, accum_init=_C0, reference=reference)
    row = 1 + len(dve_ops.OPS)
    assert row < 0x20
    dve_ops._SUB_OPCODE_FOR_NAME[name] = row
    shas = {}
    for ver in ("v3", "v4"):
        try:
            uops = _dve_lower(spec, ver=ver)
            shas[ver] = _DveOpSpec(name=name, opcode=row, uops=uops,
                                   rd1_en=True).sha(ver)
        except Exception:
            pass
    op = dve_ops.DveOp(name, spec, subdim=False, uops_sha=shas)
    dve_ops.OPS.append(op)
    dve_ops.CUSTOM_DVE_SPECS[name] = spec
    return op


def _ref_sqsum(in0, in1, c0, c1, c2):
    b = (in0.astype(np.float32) ** 2 + in1.astype(np.float32) ** 2).astype(np.float32)
    return b, c0 + b.reshape(b.shape[0], -1).sum(axis=-1, keepdims=True)


def _ref_sqadd(in0, in1, c0, c1, c2):
    b = (in0.astype(np.float32) ** 2 + in1.astype(np.float32)).astype(np.float32)
    return b, c0 + b.reshape(b.shape[0], -1).sum(axis=-1, keepdims=True)


SQSUM = _register_custom("SQSUM_ANT", _dve_sq(_Src0) + _dve_sq(_Src1), _ref_sqsum)
SQADD = _register_custom("SQADD_ANT", _dve_sq(_Src0) + _Src1, _ref_sqadd)

F32 = mybir.dt.float32
BF16 = mybir.dt.bfloat16
NP_BF16 = mybir.dt.np(BF16)

# Problem constants (hardcoded: nn_BoundaryDistillationLoss_87230785781774)
B, C, H, W = 4, 19, 512, 1024
NCORES = 8
ROWS_PER_CORE = (B * H) // NCORES          # 256
HIN = ROWS_PER_CORE + 2                    # 258 (one halo row each side)
SLABS = ((0, 128, 126), (126, 128, 126))   # (in_row_start, n_in, n_out)
REM = (252, 6)                             # packed tail rows 252..257 -> 252..255
CHUNKS = ((0, 4), (4, 4), (8, 4), (12, 4), (16, 3))
WC = 512
PCH_BUFS = 7                               # per-ti chunk-tile ring depth

NQ = 2 * C * 2 + 4                         # SQADD accum cols: (slab,cc,wh) + rem
NS = 2 + 1                                 # per-slab sqrt cols + rem sqrt col
NACC = NQ + NS


def _shifted_band(a, n, nfull=128):
    """lhsT [nfull, nfull] with lhsT[k, m] = a[m+1, k] (out row m = conv row
    m+1 so consumers start at partition 0); a is [n, n]."""
    t = np.zeros((nfull, nfull), np.float32)
    t[:n, : n - 1] = a.T[:, 1:]
    return t


def _base_bands(n):
    A_s = np.zeros((n, n), np.float32)
    A_d = np.zeros((n, n), np.float32)
    i = np.arange(n)
    A_s[i, i] = 2.0
    A_s[i[:-1], i[:-1] + 1] = 1.0
    A_s[i[1:], i[1:] - 1] = 1.0
    A_d[i[:-1], i[:-1] + 1] = 1.0
    A_d[i[1:], i[1:] - 1] = -1.0
    return A_s, A_d


def _band_weights(c_dim=C, blk=6):
    A_s, A_d = _base_bands(128)
    out = {
        "w_sp": _shifted_band(A_s, 128),
        "w_sn": _shifted_band(-A_s, 128),
        "w_d": _shifted_band(A_d, 128),
        "w_d2": _shifted_band(2.0 * A_d, 128),
        "ident": np.eye(128, dtype=np.float32),
    }
    a_s, a_d = _base_bands(blk)
    npk = c_dim * blk
    assert npk <= 128
    for name, a in (("w_rsp", a_s), ("w_rsn", -a_s), ("w_rd", a_d),
                    ("w_rd2", 2.0 * a_d)):
        m = np.zeros((128, 128), np.float32)
        sb_ = _shifted_band(a, blk, blk)
        sb_[:, blk - 2:] = 0.0
        for cblk in range(c_dim):
            m[cblk * blk: (cblk + 1) * blk, cblk * blk: (cblk + 1) * blk] = sb_
        out[name] = m
    w_sel = np.zeros((128, 128), np.float32)
    w_rep = np.zeros((128, 128), np.float32)
    for cblk in range(c_dim):
        for i in range(blk):
            w_sel[cblk * blk + i, i] = 1.0
            w_rep[i, cblk * blk + i] = 1.0
    out["w_sel"] = w_sel
    out["w_rep"] = w_rep
    return {k: v.astype(NP_BF16) for k, v in out.items()}


WNAMES = ("w_sp", "w_sn", "w_d", "w_d2", "ident",
          "w_rsp", "w_rsn", "w_rd", "w_rd2", "w_sel", "w_rep")


def build_nc():
    blk = REM[1]
    npk = C * blk

    nc = bacc.Bacc("TRN2", target_bir_lowering=False)
    xs = nc.dram_tensor("xs", [HIN, C, W], BF16, kind="ExternalInput")
    xt = nc.dram_tensor("xt", [HIN, C, W], BF16, kind="ExternalInput")
    wts = {n: nc.dram_tensor(n, [128, 128], BF16, kind="ExternalInput")
           for n in WNAMES}
    acc_out = nc.dram_tensor("acc", [128, NACC], F32, kind="ExternalOutput")

    x_dram = (xs, xt)
    EXP = mybir.ActivationFunctionType.Exp
    SQRT = mybir.ActivationFunctionType.Sqrt
    SQUARE = mybir.ActivationFunctionType.Square

    qcol = iter(range(NQ))
    scol = iter(range(NQ, NACC))

    with ExitStack() as ctx:
        tc = ctx.enter_context(tile.TileContext(nc))
        sb = ctx.enter_context(tc.tile_pool(name="sb", bufs=2))
        consts = ctx.enter_context(tc.tile_pool(name="consts", bufs=1))
        psum = ctx.enter_context(tc.tile_pool(name="psum", bufs=1, space="PSUM"))

        w_sb = {}
        for name in WNAMES:
            t = consts.tile([128, 128], BF16, tag=name)
            nc.sync.dma_start(out=t, in_=wts[name][:, :])
            w_sb[name] = t
        acc_sb = consts.tile([128, NACC], F32, tag="acc")
        nc.vector.memset(acc_sb[:, :], 0.0)

        # PE HAM warm-up: ~4us of dummy matmuls so real convs start at 2.4GHz
        warm = psum.tile([128, 1024], F32, tag="qx", bufs=2)
        for _ in range(24):
            nc.tensor.matmul(warm[:, 0:128], lhsT=w_sb["ident"][:, :],
                             rhs=w_sb["ident"][:, :], start=True, stop=True)

        MM = nc.tensor.matmul

        # ---------- build-phase helpers (per slab, tensor) ----------
        def emit_dma_exp(pmap, s, ti, ci):
            r0, nin, _ = SLABS[s]
            cc0, cn = CHUNKS[ci]
            t = sb.tile([128, 4, W + 4], BF16, tag=f"pch{ti}", bufs=PCH_BUFS)
            # zero the conv border cols (1 and W+2); cols 0 / W+3 unused
            nc.vector.memset(t[0:nin, 0:cn, 1: W + 3: W + 1], 0.0)
            nc.sync.dma_start(
                out=t[0:nin, 0:cn, 2: 2 + W],
                in_=x_dram[ti][r0: r0 + nin, cc0: cc0 + cn, :])
            nc.scalar.activation(out=t[0:nin, 0:cn, 2: 2 + W],
                                 in_=t[0:nin, 0:cn, 2: 2 + W], func=EXP)
            pmap[(ti, ci)] = t

        def emit_z_chunk(zt, pmap, s, ti, ci):
            """z-sum matmuls for one exp'd chunk; doubles as PE HAM trickle
            during the prologue (keeps the clock warm between exp batches)."""
            _, nin, _ = SLABS[s]
            cc0, cn = CHUNKS[ci]
            t = pmap[(ti, ci)]
            for wh in (0, 1):
                for c in range(cn):
                    MM(zt[0:nin, wh * WC: (wh + 1) * WC],
                       lhsT=w_sb["ident"][0:nin, 0:nin],
                       rhs=t[0:nin, c, 2 + wh * WC: 2 + (wh + 1) * WC],
                       start=(ci == 0 and c == 0), stop=(ci == 4 and c == cn - 1))

        def emit_z(pmap, s, ti):
            zt = psum.tile([128, 1024], F32, tag="qx", bufs=2)
            for ci in range(5):
                emit_z_chunk(zt, pmap, s, ti, ci)
            return zt

        def emit_recip(s, ti, zt):
            _, nin, _ = SLABS[s]
            r32 = sb.tile([128, W], F32, tag="r32", bufs=2)
            nc.vector.reciprocal_approx_fast(out=r32[0:nin, :],
                                             in_=zt[0:nin, 0:1024])
            r16 = sb.tile([128, W], BF16, tag="r16", bufs=2)
            nc.vector.tensor_copy(out=r16[0:nin, :], in_=r32[0:nin, :])
            return r16

        def emit_norm(pmap, s, ti, r16, cis):
            _, nin, _ = SLABS[s]
            for ci in cis:
                cc0, cn = CHUNKS[ci]
                t = pmap[(ti, ci)]
                for c in range(cn):
                    nc.vector.tensor_mul(out=t[0:nin, c, 2: 2 + W],
                                         in0=t[0:nin, c, 2: 2 + W],
                                         in1=r16[0:nin, :])

        # ---------- conv + extraction for one (slab, channel) ----------
        def emit_conv_cc(pmap, s, cc, mt):
            _, nin, nout = SLABS[s]
            ci, cl = cc // 4, cc % 4
            va = pmap[(0, ci)][0:nin, cl, :]
            vb = pmap[(1, ci)][0:nin, cl, :]
            for wh in (0, 1):
                b0 = wh * WC
                # gx pair in its own 2-bank tile: released by the ScalarE
                # Square alone, so 4 groups stay in flight and PE never
                # micro-idles (HAM stays at full clock)
                qx = psum.tile([128, 1024], F32, tag="qx", bufs=2)
                for w_name, off, st, sp in (
                        ("w_sp", 3, True, False), ("w_sn", 1, False, True)):
                    for k, v in ((0, va), (1, vb)):
                        MM(qx[:, k * WC: (k + 1) * WC],
                           lhsT=w_sb[w_name][0:nin, :],
                           rhs=v[:, b0 + off: b0 + off + WC],
                           start=st, stop=sp)
                qy = psum.tile([128, 1024], F32, tag="qy", bufs=2)
                for w_name, off, st, sp in (
                        ("w_d", 1, True, False), ("w_d", 3, False, False),
                        ("w_d2", 2, False, True)):
                    for k, v in ((0, va), (1, vb)):
                        MM(qy[:, k * WC: (k + 1) * WC],
                           lhsT=w_sb[w_name][0:nin, :],
                           rhs=v[:, b0 + off: b0 + off + WC],
                           start=st, stop=sp)
                g2 = sb.tile([128, 1024], BF16, tag="g2", bufs=3)
                nc.scalar.activation(out=g2[0:nout, :], in_=qx[0:nout, :],
                                     func=SQUARE)
                q = sb.tile([128, 1024], BF16, tag="q", bufs=3)
                col = next(qcol)
                nc.vector._custom_dve(
                    SQADD, out=q[0:nout, :], in0=qy[0:nout, :],
                    in1=g2[0:nout, :], s0=0.0,
                    accum_out=acc_sb[0:nout, col: col + 1])
                nc.gpsimd.tensor_mul(out=mt[0:nout, cc, b0: b0 + WC],
                                     in0=q[0:nout, 0:WC],
                                     in1=q[0:nout, WC: 2 * WC])

        # ---------- prologue: slab0 build + remainder staging ----------
        # z-sum psum slots held across the build; z matmuls trickle in per
        # exp'd chunk so the PE never idles a full HAM window.
        cur_p, nxt_p = {}, {}
        zslot0 = psum.tile([128, 1024], F32, tag="qx", bufs=2)
        zslot1 = psum.tile([128, 1024], F32, tag="qx", bufs=2)
        zslots = {0: zslot0, 1: zslot1}
        for ci in range(5):
            for ti in (0, 1):
                emit_dma_exp(cur_p, 0, ti, ci)
                emit_z_chunk(zslots[ti], cur_p, 0, ti, ci)

        # remainder inputs: packed (c, r) partitions, one DMA per tensor
        rps = sb.tile([128, 2, W + 4], BF16, tag="rps", bufs=1)
        nc.vector.memset(rps[0:npk, :, 1: W + 3: W + 1], 0.0)
        rstg = sb.tile([128, 2, W], BF16, tag="rstg", bufs=1)
        for ti in (0, 1):
            nc.sync.dma_start(
                out=rstg[0:npk, ti, :],
                in_=x_dram[ti][REM[0]: REM[0] + blk, :, :]
                .rearrange("h c w -> c h w"))
        nc.scalar.activation(out=rps[0:npk, :, 2: 2 + W],
                             in_=rstg[0:npk, :, :], func=EXP)

        # z + reciprocal + first-chunk normalize for slab0; remaining chunks'
        # normalizes are injected into the cc loop just before they're needed
        # so VectorE never lumps them ahead of the SQADD stream.
        r16s = {}
        for ti in (0, 1):
            r16s[(0, ti)] = emit_recip(0, ti, zslots[ti])
            emit_norm(cur_p, 0, ti, r16s[(0, ti)], (0,))

        # ---------- main slabs with pipelined next-slab build ----------
        for s in (0, 1):
            mt = sb.tile([128, C, W], BF16, tag="m", bufs=1)
            pops = {}

            def pin(cc, f):
                pops.setdefault(cc, []).append(f)

            def nrm(pmap, ps, ti, ci):
                return lambda: emit_norm(pmap, ps, ti, r16s[(ps, ti)], (ci,))

            # late normalizes of the current slab (chunk ci needed at cc=4*ci)
            cp = dict(cur_p)
            for ci in (1, 2, 3, 4):
                for ti in (0, 1):
                    pin(4 * ci - 4 + ti, nrm(cp, s, ti, ci))
            if s == 0:
                np_ = nxt_p

                def de(ti, ci):
                    return lambda: emit_dma_exp(np_, 1, ti, ci)

                def zr(ti):
                    def f():
                        zt = emit_z(np_, 1, ti)
                        r16s[(1, ti)] = emit_recip(1, ti, zt)
                        emit_norm(np_, 1, ti, r16s[(1, ti)], (0,))
                    return f

                for cc, f in ((2, de(0, 0)), (3, de(1, 0)), (4, de(0, 1)),
                              (5, de(1, 1)), (8, de(0, 2)), (9, de(1, 2)),
                              (12, de(0, 3)), (13, de(1, 3)), (16, de(0, 4)),
                              (17, de(1, 4)), (17, zr(0)), (18, zr(1))):
                    pin(cc, f)
            for cc in range(C):
                emit_conv_cc(cur_p, s, cc, mt)
                for f in pops.get(cc, ()):
                    f()
            sc = next(scol)
            _, _, nout = SLABS[s]
            nc.scalar.activation(out=mt[0:nout, :, :], in_=mt[0:nout, :, :],
                                 func=SQRT,
                                 accum_out=acc_sb[0:nout, sc: sc + 1])
            cur_p, nxt_p = nxt_p, {}

        # ---------- packed remainder compute ----------
        for ti in (0, 1):
            zq = psum.tile([128, 1024], F32, tag="qx", bufs=2)
            for wh in (0, 1):
                MM(zq[0:blk, wh * WC: (wh + 1) * WC],
                   lhsT=w_sb["w_sel"][0:npk, 0:blk],
                   rhs=rps[0:npk, ti, 2 + wh * WC: 2 + (wh + 1) * WC],
                   start=True, stop=True)
            r32 = sb.tile([128, W], F32, tag="r32", bufs=2)
            nc.vector.reciprocal_approx_fast(out=r32[0:blk, :],
                                             in_=zq[0:blk, 0:1024])
            r16 = sb.tile([128, W], BF16, tag="r16", bufs=2)
            nc.vector.tensor_copy(out=r16[0:blk, :], in_=r32[0:blk, :])
            rrep = psum.tile([128, 1024], F32, tag="qx", bufs=2)
            for wh in (0, 1):
                MM(rrep[0:npk, wh * WC: (wh + 1) * WC],
                   lhsT=w_sb["w_rep"][0:blk, 0:npk],
                   rhs=r16[0:blk, wh * WC: (wh + 1) * WC],
                   start=True, stop=True)
            nc.vector.tensor_mul(out=rps[0:npk, ti, 2: 2 + W],
                                 in0=rps[0:npk, ti, 2: 2 + W],
                                 in1=rrep[0:npk, 0:1024])
        mt = sb.tile([128, C, W], BF16, tag="m", bufs=1)
        q_tiles = []
        for ti in (0, 1):
            q = sb.tile([128, 1024], BF16, tag="q", bufs=3)
            for wh in (0, 1):
                b0 = wh * WC
                qxr = psum.tile([128, 1024], F32, tag="qx", bufs=2)
                for w_name, off, st, sp in (
                        ("w_rsp", 3, True, False), ("w_rsn", 1, False, True)):
                    MM(qxr[:, 0:WC], lhsT=w_sb[w_name][0:npk, :],
                       rhs=rps[0:npk, ti, b0 + off: b0 + off + WC],
                       start=st, stop=sp)
                qyr = psum.tile([128, 1024], F32, tag="qy", bufs=2)
                for w_name, off, st, sp in (
                        ("w_rd", 1, True, False), ("w_rd", 3, False, False),
                        ("w_rd2", 2, False, True)):
                    MM(qyr[:, 0:WC], lhsT=w_sb[w_name][0:npk, :],
                       rhs=rps[0:npk, ti, b0 + off: b0 + off + WC],
                       start=st, stop=sp)
                g2 = sb.tile([128, 1024], BF16, tag="g2", bufs=3)
                nc.scalar.activation(out=g2[0:npk, 0:WC], in_=qxr[0:npk, 0:WC],
                                     func=SQUARE)
                col = next(qcol)
                nc.vector._custom_dve(
                    SQADD, out=q[0:npk, b0: b0 + WC],
                    in0=qyr[0:npk, 0:WC], in1=g2[0:npk, 0:WC], s0=0.0,
                    accum_out=acc_sb[0:npk, col: col + 1])
            q_tiles.append(q)
        nc.gpsimd.tensor_mul(out=mt[0:npk, 0, :], in0=q_tiles[0][0:npk, :],
                             in1=q_tiles[1][0:npk, :])
        sc = next(scol)
        nc.scalar.activation(out=mt[0:npk, 0, :], in_=mt[0:npk, 0, :],
                             func=SQRT, accum_out=acc_sb[0:npk, sc: sc + 1])

        nc.sync.dma_start(out=acc_out[:, :], in_=acc_sb[:, :])
    if not nc.is_finalized():
        nc.finalize()
    return nc


def shard_inputs(student_logits, teacher_logits, c_dim=C, h_dim=H, w_dim=W,
                 ncores=NCORES):
    """Full (B,C,H,W) fp32 -> per-core (rows+2, C, W) bf16 halo shards."""
    b_dim = student_logits.shape[0]
    rows = (b_dim * h_dim) // ncores
    wts = _band_weights()
    in_maps = []
    for k in range(ncores):
        g0 = k * rows
        bi, h0 = g0 // h_dim, g0 % h_dim
        m = {}
        for name, x in (("xs", student_logits), ("xt", teacher_logits)):
            img = np.asarray(x[bi], np.float32)            # (C, H, W)
            sh = np.zeros((rows + 2, c_dim, w_dim), NP_BF16)
            lo, hi = h0 - 1, h0 + rows + 1
            slo, shi = max(lo, 0), min(hi, h_dim)
            sh[slo - lo: slo - lo + (shi - slo)] = \
                img[:, slo:shi, :].transpose(1, 0, 2).astype(NP_BF16)
            m[name] = sh
        for name, wv in wts.items():
            m[name] = wv
        in_maps.append(m)
    return in_maps


_NC_CACHE = {}


def _get_nc():
    if "full" not in _NC_CACHE:
        _NC_CACHE["full"] = build_nc()
    return _NC_CACHE["full"]


def run_on_cores(in_maps, trace=False, **kw):
    nc = _get_nc()
    return bass_utils.run_bass_kernel_spmd(
        nc, in_maps, core_ids=list(range(len(in_maps))), trace=trace, **kw)


def finish(results, n_total=None):
    if n_total is None:
        n_total = B * C * H * W
    tq = 0.0
    tcross = 0.0
    for r in results:
        a = np.asarray(r["acc"], np.float64)
        tq += a[:, :NQ].sum()
        tcross += a[:, NQ:].sum()
    return np.float32((tq - 2.0 * tcross) / n_total)


def kernel(student_logits, teacher_logits):
    in_maps = shard_inputs(np.asarray(student_logits), np.asarray(teacher_logits))
    res = run_on_cores(in_maps)
    return finish(res.results)


# revision 15
# speedup vs baseline: 1.1842x; 1.1842x over previous
"""Trainium2 Bass kernel for nn_BoundaryDistillationLoss.

loss = mean((|grad(softmax(s))| - |grad(softmax(t))|)^2) with depthwise 3x3
Sobel gradients, expanded as [ sum(qs) + sum(qt) - 2*sum(sqrt(qs*qt)) ] / N
with q = gx^2 + gy^2.

Host pre-casts inputs to bf16 and rearranges each core's shard to (h, c, w)
so every DMA is contiguous.  2048 rows data-parallel over 8 cores; per core
two 128-row slabs (126 output rows each) plus a packed 6-row tail that is
interleaved into slab 1.  On-chip: h-rows on partitions, (c, w) on the free
dim, per-4-channel chunk tiles in a ring so slab N+1's DMA/exp/z-sum/
normalize overlaps slab N's conv.  Sobel row-taps are banded 128x128
matmuls emitted weight-major (one LDWEIGHTS per band per channel, both
tensors and both w-halves share it); col-taps are +-1-shifted rhs views of a
zero-padded slab.  gx pairs land in 2-bank psum tiles released by a square
pass (split ~2:1 between ScalarE Square and a fused VectorE SQSUM against a
zeros tile to balance those engines); gy pairs are consumed by a fused
VectorE square-add with free accumulation; GPSIMD multiplies qs*qt; one big
in-place SQRT per slab avoids exp<->sqrt activation-table thrashing.  The
z-sum matmuls double as PE clock-warming trickle during the prologue.
"""

import numpy as np
from contextlib import ExitStack

import concourse.bass as bass
import concourse.bacc as bacc
import concourse.mybir as mybir
import concourse.tile as tile
from concourse import bass_utils
import concourse.dve_ops as dve_ops
from concourse.dve_spec import C0 as _C0, Spec as _Spec, Src0 as _Src0, \
    Src1 as _Src1, lower as _dve_lower, sq as _dve_sq
from concourse.dve_uop import DveOpSpec as _DveOpSpec
from operator import add as _op_add


def _register_custom(name, body, reference):
    for o in dve_ops.OPS:
        if o.name == name:
            return o
    spec = _Spec(body=body, accum=_op_add, accum_init=_C0, reference=reference)
    row = 1 + len(dve_ops.OPS)
    assert row < 0x20
    dve_ops._SUB_OPCODE_FOR_NAME[name] = row
    shas = {}
    for ver in ("v3", "v4"):
        try:
            uops = _dve_lower(spec, ver=ver)
            shas[ver] = _DveOpSpec(name=name, opcode=row, uops=uops,
                                   rd1_en=True).sha(ver)
        except Exception:
            pass
    op = dve_ops.DveOp(name, spec, subdim=False, uops_sha=shas)
    dve_ops.OPS.append(op)
    dve_ops.CUSTOM_DVE_SPECS[name] = spec
    return op


def _ref_sqsum(in0, in1, c0, c1, c2):
    b = (in0.astype(np.float32) ** 2 + in1.astype(np.float32) ** 2).astype(np.float32)
    return b, c0 + b.reshape(b.shape[0], -1).sum(axis=-1, keepdims=True)


def _ref_sqadd(in0, in1, c0, c1, c2):
    b = (in0.astype(np.float32) ** 2 + in1.astype(np.float32)).astype(np.float32)
    return b, c0 + b.reshape(b.shape[0], -1).sum(axis=-1, keepdims=True)


SQSUM = _register_custom("SQSUM_ANT", _dve_sq(_Src0) + _dve_sq(_Src1), _ref_sqsum)
SQADD = _register_custom("SQADD_ANT", _dve_sq(_Src0) + _Src1, _ref_sqadd)

F32 = mybir.dt.float32
BF16 = mybir.dt.bfloat16
NP_BF16 = mybir.dt.np(BF16)

# Problem constants (hardcoded: nn_BoundaryDistillationLoss_87230785781774)
B, C, H, W = 4, 19, 512, 1024
NCORES = 8
ROWS_PER_CORE = (B * H) // NCORES          # 256
HIN = ROWS_PER_CORE + 2                    # 258 (one halo row each side)
SLABS = ((0, 128, 126), (126, 128, 126))   # (in_row_start, n_in, n_out)
REM = (252, 6)                             # packed tail rows 252..257 -> 252..255
CHUNKS = ((0, 4), (4, 4), (8, 4), (12, 4), (16, 3))
WC = 512
PCH_BUFS = 7                               # per-ti chunk-tile ring depth

NQ = 2 * C * 2 + 4                         # SQADD accum cols: (slab,cc,wh) + rem
NS = 2 + 1                                 # per-slab sqrt cols + rem sqrt col
NACC = NQ + NS + 1                         # +1 junk col for unused accums
JCOL = NQ + NS


def _shifted_band(a, n, nfull=128):
    """lhsT [nfull, nfull] with lhsT[k, m] = a[m+1, k] (out row m = conv row
    m+1 so consumers start at partition 0); a is [n, n]."""
    t = np.zeros((nfull, nfull), np.float32)
    t[:n, : n - 1] = a.T[:, 1:]
    return t


def _base_bands(n):
    A_s = np.zeros((n, n), np.float32)
    A_d = np.zeros((n, n), np.float32)
    i = np.arange(n)
    A_s[i, i] = 2.0
    A_s[i[:-1], i[:-1] + 1] = 1.0
    A_s[i[1:], i[1:] - 1] = 1.0
    A_d[i[:-1], i[:-1] + 1] = 1.0
    A_d[i[1:], i[1:] - 1] = -1.0
    return A_s, A_d


WNAMES = ("w_sp", "w_sn", "w_d", "w_d2", "ident",
          "w_rsp", "w_rsn", "w_rd", "w_rd2", "w_sel", "w_rep")


def _band_weights(c_dim=C, blk=6):
    A_s, A_d = _base_bands(128)
    out = {
        "w_sp": _shifted_band(A_s, 128),
        "w_sn": _shifted_band(-A_s, 128),
        "w_d": _shifted_band(A_d, 128),
        "w_d2": _shifted_band(2.0 * A_d, 128),
        "ident": np.eye(128, dtype=np.float32),
    }
    a_s, a_d = _base_bands(blk)
    npk = c_dim * blk
    assert npk <= 128
    for name, a in (("w_rsp", a_s), ("w_rsn", -a_s), ("w_rd", a_d),
                    ("w_rd2", 2.0 * a_d)):
        m = np.zeros((128, 128), np.float32)
        sb_ = _shifted_band(a, blk, blk)
        sb_[:, blk - 2:] = 0.0
        for cblk in range(c_dim):
            m[cblk * blk: (cblk + 1) * blk, cblk * blk: (cblk + 1) * blk] = sb_
        out[name] = m
    w_sel = np.zeros((128, 128), np.float32)
    w_rep = np.zeros((128, 128), np.float32)
    for cblk in range(c_dim):
        for i in range(blk):
            w_sel[cblk * blk + i, i] = 1.0
            w_rep[i, cblk * blk + i] = 1.0
    out["w_sel"] = w_sel
    out["w_rep"] = w_rep
    return np.concatenate([out[n].astype(NP_BF16) for n in WNAMES], axis=1)


def build_nc():
    blk = REM[1]
    npk = C * blk

    nc = bacc.Bacc("TRN2", target_bir_lowering=False)
    xs = nc.dram_tensor("xs", [HIN, C, W], BF16, kind="ExternalInput")
    xt = nc.dram_tensor("xt", [HIN, C, W], BF16, kind="ExternalInput")
    wpack = nc.dram_tensor("wpack", [128, 128 * len(WNAMES)], BF16,
                           kind="ExternalInput")
    acc_out = nc.dram_tensor("acc", [128, NACC], F32, kind="ExternalOutput")

    x_dram = (xs, xt)
    EXP = mybir.ActivationFunctionType.Exp
    SQRT = mybir.ActivationFunctionType.Sqrt
    SQUARE = mybir.ActivationFunctionType.Square

    qcol = iter(range(NQ))
    scol = iter(range(NQ, NQ + NS))

    with ExitStack() as ctx:
        tc = ctx.enter_context(tile.TileContext(nc))
        sb = ctx.enter_context(tc.tile_pool(name="sb", bufs=2))
        consts = ctx.enter_context(tc.tile_pool(name="consts", bufs=1))
        psum = ctx.enter_context(tc.tile_pool(name="psum", bufs=1, space="PSUM"))

        wall = consts.tile([128, 128 * len(WNAMES)], BF16, tag="wall")
        nc.sync.dma_start(out=wall, in_=wpack[:, :])
        w_sb = {n: wall[:, i * 128: (i + 1) * 128]
                for i, n in enumerate(WNAMES)}
        acc_sb = consts.tile([128, NACC], F32, tag="acc")
        nc.vector.memset(acc_sb[:, :], 0.0)
        zeros = consts.tile([128, 1024], BF16, tag="zeros")
        nc.vector.memset(zeros[:, :], 0.0)
        jk = acc_sb[:, JCOL: JCOL + 1]

        # PE HAM warm-up: ~4us of dummy matmuls so real convs start at 2.4GHz
        warm = psum.tile([128, 1024], F32, tag="qx", bufs=2)
        for _ in range(24):
            nc.tensor.matmul(warm[:, 0:128], lhsT=w_sb["ident"][:, :],
                             rhs=w_sb["ident"][:, :], start=True, stop=True)

        MM = nc.tensor.matmul

        # ---------- build-phase helpers (per slab, tensor) ----------
        def emit_dma_exp(pmap, s, ti, ci):
            r0, nin, _ = SLABS[s]
            cc0, cn = CHUNKS[ci]
            t = sb.tile([128, 4, W + 4], BF16, tag=f"pch{ti}", bufs=PCH_BUFS)
            # zero the conv border cols (1 and W+2); cols 0 / W+3 unused
            nc.vector.memset(t[0:nin, 0:cn, 1: W + 3: W + 1], 0.0)
            nc.sync.dma_start(
                out=t[0:nin, 0:cn, 2: 2 + W],
                in_=x_dram[ti][r0: r0 + nin, cc0: cc0 + cn, :])
            nc.scalar.activation(out=t[0:nin, 0:cn, 2: 2 + W],
                                 in_=t[0:nin, 0:cn, 2: 2 + W], func=EXP)
            pmap[(ti, ci)] = t

        def emit_z_chunk(zt, pmap, s, ti, ci):
            """z-sum matmuls for one exp'd chunk; doubles as PE HAM trickle
            during the prologue (keeps the clock warm between exp batches)."""
            _, nin, _ = SLABS[s]
            cc0, cn = CHUNKS[ci]
            t = pmap[(ti, ci)]
            for wh in (0, 1):
                for c in range(cn):
                    MM(zt[0:nin, wh * WC: (wh + 1) * WC],
                       lhsT=w_sb["ident"][0:nin, 0:nin],
                       rhs=t[0:nin, c, 2 + wh * WC: 2 + (wh + 1) * WC],
                       start=(ci == 0 and c == 0), stop=(ci == 4 and c == cn - 1))

        def emit_z(pmap, s, ti):
            zt = psum.tile([128, 1024], F32, tag="qx", bufs=2)
            for ci in range(5):
                emit_z_chunk(zt, pmap, s, ti, ci)
            return zt

        def emit_recip(s, ti, zt):
            _, nin, _ = SLABS[s]
            r32 = sb.tile([128, W], F32, tag="r32", bufs=2)
            nc.vector.reciprocal_approx_fast(out=r32[0:nin, :],
                                             in_=zt[0:nin, 0:1024])
            r16 = sb.tile([128, W], BF16, tag="r16", bufs=2)
            nc.vector.tensor_copy(out=r16[0:nin, :], in_=r32[0:nin, :])
            return r16

        def emit_norm(pmap, s, ti, r16, cis):
            _, nin, _ = SLABS[s]
            for ci in cis:
                cc0, cn = CHUNKS[ci]
                t = pmap[(ti, ci)]
                for c in range(cn):
                    nc.vector.tensor_mul(out=t[0:nin, c, 2: 2 + W],
                                         in0=t[0:nin, c, 2: 2 + W],
                                         in1=r16[0:nin, :])

        # ---------- conv + extraction for one (slab, channel) ----------
        def emit_conv_cc(pmap, s, cc, mt):
            _, nin, nout = SLABS[s]
            ci, cl = cc // 4, cc % 4
            va = pmap[(0, ci)][0:nin, cl, :]
            vb = pmap[(1, ci)][0:nin, cl, :]
            # weight-major: each band matrix is loaded once per channel and
            # serves both tensors and both w-halves (4 LDWEIGHTS per channel)
            qx0 = psum.tile([128, 1024], F32, tag="qx", bufs=2)
            qx1 = psum.tile([128, 1024], F32, tag="qx", bufs=2)
            qx = (qx0, qx1)
            for w_name, st, sp, off in (("w_sp", True, False, 3),
                                        ("w_sn", False, True, 1)):
                for wh in (0, 1):
                    b0 = wh * WC
                    for k, v in ((0, va), (1, vb)):
                        MM(qx[wh][:, k * WC: (k + 1) * WC],
                           lhsT=w_sb[w_name][0:nin, :],
                           rhs=v[:, b0 + off: b0 + off + WC],
                           start=st, stop=sp)
            qy0 = psum.tile([128, 1024], F32, tag="qy", bufs=2)
            qy1 = psum.tile([128, 1024], F32, tag="qy", bufs=2)
            qy = (qy0, qy1)
            for w_name, st, sp, offs in (("w_d", True, False, (1, 3)),
                                         ("w_d2", False, True, (2,))):
                for off in offs:
                    for wh in (0, 1):
                        b0 = wh * WC
                        for k, v in ((0, va), (1, vb)):
                            MM(qy[wh][:, k * WC: (k + 1) * WC],
                               lhsT=w_sb[w_name][0:nin, :],
                               rhs=v[:, b0 + off: b0 + off + WC],
                               start=(st and off == 1), stop=sp)
            for wh in (0, 1):
                b0 = wh * WC
                g2 = sb.tile([128, 1024], BF16, tag="g2", bufs=3)
                if (cc * 2 + wh) % 3 < 2:
                    nc.scalar.activation(out=g2[0:nout, :],
                                         in_=qx[wh][0:nout, :], func=SQUARE)
                else:
                    nc.vector._custom_dve(
                        SQSUM, out=g2[0:nout, :], in0=qx[wh][0:nout, :],
                        in1=zeros[0:nout, :], s0=0.0, accum_out=jk[0:nout, :])
                q = sb.tile([128, 1024], BF16, tag="q", bufs=3)
                col = next(qcol)
                nc.vector._custom_dve(
                    SQADD, out=q[0:nout, :], in0=qy[wh][0:nout, :],
                    in1=g2[0:nout, :], s0=0.0,
                    accum_out=acc_sb[0:nout, col: col + 1])
                nc.gpsimd.tensor_mul(out=mt[0:nout, cc, b0: b0 + WC],
                                     in0=q[0:nout, 0:WC],
                                     in1=q[0:nout, WC: 2 * WC])

        # ---------- prologue: slab0 build + remainder staging ----------
        # z-sum psum slots held across the build; z matmuls trickle in per
        # exp'd chunk so the PE never idles a full HAM window.
        cur_p, nxt_p = {}, {}
        zslot0 = psum.tile([128, 1024], F32, tag="qx", bufs=2)
        zslot1 = psum.tile([128, 1024], F32, tag="qx", bufs=2)
        zslots = {0: zslot0, 1: zslot1}
        for ci in range(5):
            for ti in (0, 1):
                emit_dma_exp(cur_p, 0, ti, ci)
                emit_z_chunk(zslots[ti], cur_p, 0, ti, ci)

        # remainder inputs: packed (c, r) partitions, one DMA per tensor
        rps = sb.tile([128, 2, W + 4], BF16, tag="rps", bufs=1)
        nc.vector.memset(rps[0:npk, :, 1: W + 3: W + 1], 0.0)
        rstg = sb.tile([128, 2, W], BF16, tag="rstg", bufs=1)
        for ti in (0, 1):
            nc.sync.dma_start(
                out=rstg[0:npk, ti, :],
                in_=x_dram[ti][REM[0]: REM[0] + blk, :, :]
                .rearrange("h c w -> c h w"))
        nc.scalar.activation(out=rps[0:npk, :, 2: 2 + W],
                             in_=rstg[0:npk, :, :], func=EXP)

        r16s = {}
        for ti in (0, 1):
            r16s[(0, ti)] = emit_recip(0, ti, zslots[ti])
            emit_norm(cur_p, 0, ti, r16s[(0, ti)], (0,))

        # ---------- remainder compute closures (interleaved into slab 1) ----
        rqs = {}

        def rem_build(ti):
            def f():
                zq = psum.tile([128, 1024], F32, tag="qx", bufs=2)
                for wh in (0, 1):
                    MM(zq[0:blk, wh * WC: (wh + 1) * WC],
                       lhsT=w_sb["w_sel"][0:npk, 0:blk],
                       rhs=rps[0:npk, ti, 2 + wh * WC: 2 + (wh + 1) * WC],
                       start=True, stop=True)
                r32 = sb.tile([128, W], F32, tag="r32", bufs=2)
                nc.vector.reciprocal_approx_fast(out=r32[0:blk, :],
                                                 in_=zq[0:blk, 0:1024])
                r16 = sb.tile([128, W], BF16, tag="r16", bufs=2)
                nc.vector.tensor_copy(out=r16[0:blk, :], in_=r32[0:blk, :])
                rrep = psum.tile([128, 1024], F32, tag="qx", bufs=2)
                for wh in (0, 1):
                    MM(rrep[0:npk, wh * WC: (wh + 1) * WC],
                       lhsT=w_sb["w_rep"][0:blk, 0:npk],
                       rhs=r16[0:blk, wh * WC: (wh + 1) * WC],
                       start=True, stop=True)
                nc.vector.tensor_mul(out=rps[0:npk, ti, 2: 2 + W],
                                     in0=rps[0:npk, ti, 2: 2 + W],
                                     in1=rrep[0:npk, 0:1024])
            return f

        def rem_conv(ti):
            def f():
                rq = sb.tile([128, 1024], BF16, tag="rq", bufs=3)
                rqs[ti] = rq
                for wh in (0, 1):
                    b0 = wh * WC
                    qxr = psum.tile([128, 1024], F32, tag="qx", bufs=2)
                    for w_name, st, sp in (("w_rsp", True, False),
                                           ("w_rsn", False, True)):
                        off = 3 if st else 1
                        MM(qxr[:, 0:WC], lhsT=w_sb[w_name][0:npk, :],
                           rhs=rps[0:npk, ti, b0 + off: b0 + off + WC],
                           start=st, stop=sp)
                    qyr = psum.tile([128, 1024], F32, tag="qy", bufs=2)
                    for w_name, off, st, sp in (
                            ("w_rd", 1, True, False), ("w_rd", 3, False, False),
                            ("w_rd2", 2, False, True)):
                        MM(qyr[:, 0:WC], lhsT=w_sb[w_name][0:npk, :],
                           rhs=rps[0:npk, ti, b0 + off: b0 + off + WC],
                           start=st, stop=sp)
                    g2 = sb.tile([128, 1024], BF16, tag="g2", bufs=3)
                    nc.scalar.activation(out=g2[0:npk, 0:WC],
                                         in_=qxr[0:npk, 0:WC], func=SQUARE)
                    col = next(qcol)
                    nc.vector._custom_dve(
                        SQADD, out=rq[0:npk, b0: b0 + WC],
                        in0=qyr[0:npk, 0:WC], in1=g2[0:npk, 0:WC], s0=0.0,
                        accum_out=acc_sb[0:npk, col: col + 1])
            return f

        def rem_tail():
            rm = sb.tile([128, 1024], BF16, tag="rq", bufs=3)
            nc.gpsimd.tensor_mul(out=rm[0:npk, :], in0=rqs[0][0:npk, :],
                                 in1=rqs[1][0:npk, :])
            sc = next(scol)
            nc.scalar.activation(out=rm[0:npk, :], in_=rm[0:npk, :],
                                 func=SQRT, accum_out=acc_sb[0:npk, sc: sc + 1])

        # ---------- main slabs with pipelined next-slab build ----------
        for s in (0, 1):
            mt = sb.tile([128, C, W], BF16, tag="m", bufs=1)
            pops = {}

            def pin(cc, f):
                pops.setdefault(cc, []).append(f)

            def nrm(pmap, ps, ti, ci):
                return lambda: emit_norm(pmap, ps, ti, r16s[(ps, ti)], (ci,))

            # late normalizes of the current slab (chunk ci needed at cc=4*ci)
            cp = dict(cur_p)
            for ci in (1, 2, 3, 4):
                for ti in (0, 1):
                    pin(4 * ci - 4 + ti, nrm(cp, s, ti, ci))
            if s == 0:
                np_ = nxt_p

                def de(ti, ci):
                    return lambda: emit_dma_exp(np_, 1, ti, ci)

                def zr(ti):
                    def f():
                        zt = emit_z(np_, 1, ti)
                        r16s[(1, ti)] = emit_recip(1, ti, zt)
                        emit_norm(np_, 1, ti, r16s[(1, ti)], (0,))
                    return f

                for cc, f in ((2, de(0, 0)), (3, de(1, 0)), (4, de(0, 1)),
                              (5, de(1, 1)), (8, de(0, 2)), (9, de(1, 2)),
                              (12, de(0, 3)), (13, de(1, 3)), (16, de(0, 4)),
                              (17, de(1, 4)), (17, zr(0)), (18, zr(1))):
                    pin(cc, f)
            else:
                pin(2, rem_build(0))
                pin(4, rem_build(1))
                pin(6, rem_conv(0))
                pin(8, rem_conv(1))
                pin(11, rem_tail)
            for cc in range(C):
                emit_conv_cc(cur_p, s, cc, mt)
                for f in pops.get(cc, ()):
                    f()
            sc = next(scol)
            _, _, nout = SLABS[s]
            nc.scalar.activation(out=mt[0:nout, :, :], in_=mt[0:nout, :, :],
                                 func=SQRT,
                                 accum_out=acc_sb[0:nout, sc: sc + 1])
            cur_p, nxt_p = nxt_p, {}

        nc.sync.dma_start(out=acc_out[:, :], in_=acc_sb[:, :])
    if not nc.is_finalized():
        nc.finalize()
    return nc


def shard_inputs(student_logits, teacher_logits, c_dim=C, h_dim=H, w_dim=W,
                 ncores=NCORES):
    """Full (B,C,H,W) fp32 -> per-core (rows+2, C, W) bf16 halo shards."""
    b_dim = student_logits.shape[0]
    rows = (b_dim * h_dim) // ncores
    wp = _band_weights()
    in_maps = []
    for k in range(ncores):
        g0 = k * rows
        bi, h0 = g0 // h_dim, g0 % h_dim
        m = {"wpack": wp}
        for name, x in (("xs", student_logits), ("xt", teacher_logits)):
            img = np.asarray(x[bi], np.float32)            # (C, H, W)
            sh = np.zeros((rows + 2, c_dim, w_dim), NP_BF16)
            lo, hi = h0 - 1, h0 + rows + 1
            slo, shi = max(lo, 0), min(hi, h_dim)
            sh[slo - lo: slo - lo + (shi - slo)] = \
                img[:, slo:shi, :].transpose(1, 0, 2).astype(NP_BF16)
            m[name] = sh
        in_maps.append(m)
    return in_maps


_NC_CACHE = {}


def _get_nc():
    if "full" not in _NC_CACHE:
        _NC_CACHE["full"] = build_nc()
    return _NC_CACHE["full"]


def run_on_cores(in_maps, trace=False, **kw):
    nc = _get_nc()
    return bass_utils.run_bass_kernel_spmd(
        nc, in_maps, core_ids=list(range(len(in_maps))), trace=trace, **kw)


def finish(results, n_total=None):
    if n_total is None:
        n_total = B * C * H * W
    tq = 0.0
    tcross = 0.0
    for r in results:
        a = np.asarray(r["acc"], np.float64)
        tq += a[:, :NQ].sum()
        tcross += a[:, NQ: NQ + NS].sum()
    return np.float32((tq - 2.0 * tcross) / n_total)


def kernel(student_logits, teacher_logits):
    in_maps = shard_inputs(np.asarray(student_logits), np.asarray(teacher_logits))
    res = run_on_cores(in_maps)
    return finish(res.results)


# revision 22
# speedup vs baseline: 1.2048x; 1.0175x over previous
"""Trainium2 Bass kernel for nn_BoundaryDistillationLoss.

loss = mean((|grad(softmax(s))| - |grad(softmax(t))|)^2) with depthwise 3x3
Sobel gradients, expanded as [ sum(qs) + sum(qt) - 2*sum(sqrt(qs*qt)) ] / N
with q = gx^2 + gy^2.

Host pre-casts inputs to bf16 and rearranges each core's shard to (h, c, w)
so every DMA is contiguous.  2048 rows data-parallel over 8 cores; per core
two 128-row slabs (126 output rows each) plus a packed 6-row tail that is
interleaved into slab 1.  On-chip: h-rows on partitions, (c, w) on the free
dim, per-4-channel chunk tiles in a ring so slab N+1's DMA/exp/z-sum/
normalize overlaps slab N's conv.  Sobel row-taps are banded 128x128
matmuls emitted weight-major (one LDWEIGHTS per band per channel, both
tensors and both w-halves share it); col-taps are +-1-shifted rhs views of a
zero-padded slab.  gx pairs land in 2-bank psum tiles released by a square
pass (split ~2:1 between ScalarE Square and a fused VectorE SQSUM against a
zeros tile to balance those engines); gy pairs are consumed by a fused
VectorE square-add with free accumulation; GPSIMD multiplies qs*qt; one big
in-place SQRT per slab avoids exp<->sqrt activation-table thrashing.  The
z-sum matmuls double as PE clock-warming trickle during the prologue.
"""

import numpy as np
from contextlib import ExitStack

import concourse.bass as bass
import concourse.bacc as bacc
import concourse.mybir as mybir
import concourse.tile as tile
from concourse import bass_utils
import concourse.dve_ops as dve_ops
from concourse.dve_spec import C0 as _C0, Spec as _Spec, Src0 as _Src0, \
    Src1 as _Src1, lower as _dve_lower, sq as _dve_sq
from concourse.dve_uop import DveOpSpec as _DveOpSpec
from operator import add as _op_add


def _register_custom(name, body, reference):
    for o in dve_ops.OPS:
        if o.name == name:
            return o
    spec = _Spec(body=body, accum=_op_add, accum_init=_C0, reference=reference)
    row = 1 + len(dve_ops.OPS)
    assert row < 0x20
    dve_ops._SUB_OPCODE_FOR_NAME[name] = row
    shas = {}
    for ver in ("v3", "v4"):
        try:
            uops = _dve_lower(spec, ver=ver)
            shas[ver] = _DveOpSpec(name=name, opcode=row, uops=uops,
                                   rd1_en=True).sha(ver)
        except Exception:
            pass
    op = dve_ops.DveOp(name, spec, subdim=False, uops_sha=shas)
    dve_ops.OPS.append(op)
    dve_ops.CUSTOM_DVE_SPECS[name] = spec
    return op


def _ref_sqsum(in0, in1, c0, c1, c2):
    b = (in0.astype(np.float32) ** 2 + in1.astype(np.float32) ** 2).astype(np.float32)
    return b, c0 + b.reshape(b.shape[0], -1).sum(axis=-1, keepdims=True)


def _ref_sqadd(in0, in1, c0, c1, c2):
    b = (in0.astype(np.float32) ** 2 + in1.astype(np.float32)).astype(np.float32)
    return b, c0 + b.reshape(b.shape[0], -1).sum(axis=-1, keepdims=True)


SQSUM = _register_custom("SQSUM_ANT", _dve_sq(_Src0) + _dve_sq(_Src1), _ref_sqsum)
SQADD = _register_custom("SQADD_ANT", _dve_sq(_Src0) + _Src1, _ref_sqadd)

F32 = mybir.dt.float32
BF16 = mybir.dt.bfloat16
NP_BF16 = mybir.dt.np(BF16)

# Problem constants (hardcoded: nn_BoundaryDistillationLoss_87230785781774)
B, C, H, W = 4, 19, 512, 1024
NCORES = 8
ROWS_PER_CORE = (B * H) // NCORES          # 256
HIN = ROWS_PER_CORE + 2                    # 258 (one halo row each side)
SLABS = ((0, 128, 126), (126, 128, 126))   # (in_row_start, n_in, n_out)
REM = (252, 6)                             # packed tail rows 252..257 -> 252..255
CHUNKS = ((0, 4), (4, 4), (8, 4), (12, 4), (16, 3))
WC = 512
PCH_BUFS = 7                               # per-ti chunk-tile ring depth

NQ = 2 * C * 2 + 4                         # SQADD accum cols: (slab,cc,wh) + rem
NS = 2 * 2 + 1                             # split per-slab sqrt cols + rem col
NACC = NQ + NS + 1                         # +1 junk col for unused accums
JCOL = NQ + NS


def _shifted_band(a, n, nfull=128):
    """lhsT [nfull, nfull] with lhsT[k, m] = a[m+1, k] (out row m = conv row
    m+1 so consumers start at partition 0); a is [n, n]."""
    t = np.zeros((nfull, nfull), np.float32)
    t[:n, : n - 1] = a.T[:, 1:]
    return t


def _base_bands(n):
    A_s = np.zeros((n, n), np.float32)
    A_d = np.zeros((n, n), np.float32)
    i = np.arange(n)
    A_s[i, i] = 2.0
    A_s[i[:-1], i[:-1] + 1] = 1.0
    A_s[i[1:], i[1:] - 1] = 1.0
    A_d[i[:-1], i[:-1] + 1] = 1.0
    A_d[i[1:], i[1:] - 1] = -1.0
    return A_s, A_d


WNAMES = ("w_sp", "w_sn", "w_d", "w_d2", "ident",
          "w_rsp", "w_rsn", "w_rd", "w_rd2", "w_sel", "w_rep")


def _band_weights(c_dim=C, blk=6):
    A_s, A_d = _base_bands(128)
    out = {
        "w_sp": _shifted_band(A_s, 128),
        "w_sn": _shifted_band(-A_s, 128),
        "w_d": _shifted_band(A_d, 128),
        "w_d2": _shifted_band(2.0 * A_d, 128),
        "ident": np.eye(128, dtype=np.float32),
    }
    a_s, a_d = _base_bands(blk)
    npk = c_dim * blk
    assert npk <= 128
    for name, a in (("w_rsp", a_s), ("w_rsn", -a_s), ("w_rd", a_d),
                    ("w_rd2", 2.0 * a_d)):
        m = np.zeros((128, 128), np.float32)
        sb_ = _shifted_band(a, blk, blk)
        sb_[:, blk - 2:] = 0.0
        for cblk in range(c_dim):
            m[cblk * blk: (cblk + 1) * blk, cblk * blk: (cblk + 1) * blk] = sb_
        out[name] = m
    w_sel = np.zeros((128, 128), np.float32)
    w_rep = np.zeros((128, 128), np.float32)
    for cblk in range(c_dim):
        for i in range(blk):
            w_sel[cblk * blk + i, i] = 1.0
            w_rep[i, cblk * blk + i] = 1.0
    out["w_sel"] = w_sel
    out["w_rep"] = w_rep
    return np.concatenate([out[n].astype(NP_BF16) for n in WNAMES], axis=1)


def build_nc():
    blk = REM[1]
    npk = C * blk

    nc = bacc.Bacc("TRN2", target_bir_lowering=False)
    xs = nc.dram_tensor("xs", [HIN, C, W], BF16, kind="ExternalInput")
    xt = nc.dram_tensor("xt", [HIN, C, W], BF16, kind="ExternalInput")
    wpack = nc.dram_tensor("wpack", [128, 128 * len(WNAMES)], BF16,
                           kind="ExternalInput")
    acc_out = nc.dram_tensor("acc", [128, NACC], F32, kind="ExternalOutput")

    x_dram = (xs, xt)
    EXP = mybir.ActivationFunctionType.Exp
    SQRT = mybir.ActivationFunctionType.Sqrt
    SQUARE = mybir.ActivationFunctionType.Square

    qcol = iter(range(NQ))
    scol = iter(range(NQ, NQ + NS))

    with ExitStack() as ctx:
        tc = ctx.enter_context(tile.TileContext(nc))
        sb = ctx.enter_context(tc.tile_pool(name="sb", bufs=2))
        consts = ctx.enter_context(tc.tile_pool(name="consts", bufs=1))
        psum = ctx.enter_context(tc.tile_pool(name="psum", bufs=1, space="PSUM"))

        wall = consts.tile([128, 128 * len(WNAMES)], BF16, tag="wall")
        nc.sync.dma_start(out=wall, in_=wpack[:, :])
        w_sb = {n: wall[:, i * 128: (i + 1) * 128]
                for i, n in enumerate(WNAMES)}
        acc_sb = consts.tile([128, NACC], F32, tag="acc")
        nc.vector.memset(acc_sb[:, :], 0.0)
        zeros = consts.tile([128, 1024], BF16, tag="zeros")
        nc.vector.memset(zeros[:, :], 0.0)
        jk = acc_sb[:, JCOL: JCOL + 1]

        MM = nc.tensor.matmul

        def warm_mms(rhs, n):
            # dummy matmuls that depend on freshly-exp'd data: paced through
            # the prologue they keep the PE HAM activity window non-idle
            wp = psum.tile([128, 1024], F32, tag="qy", bufs=2)
            for _ in range(n):
                MM(wp[:, 0:WC], lhsT=w_sb["ident"][:, :], rhs=rhs,
                   start=True, stop=True)

        # ---------- build-phase helpers (per slab, tensor) ----------
        def emit_dma_exp(pmap, s, ti, ci):
            r0, nin, _ = SLABS[s]
            cc0, cn = CHUNKS[ci]
            t = sb.tile([128, 4, W + 4], BF16, tag=f"pch{ti}", bufs=PCH_BUFS)
            # zero the conv border cols (1 and W+2); cols 0 / W+3 unused
            nc.vector.memset(t[0:nin, 0:cn, 1: W + 3: W + 1], 0.0)
            nc.sync.dma_start(
                out=t[0:nin, 0:cn, 2: 2 + W],
                in_=x_dram[ti][r0: r0 + nin, cc0: cc0 + cn, :])
            nc.scalar.activation(out=t[0:nin, 0:cn, 2: 2 + W],
                                 in_=t[0:nin, 0:cn, 2: 2 + W], func=EXP)
            pmap[(ti, ci)] = t

        def emit_z_chunk(zt, pmap, s, ti, ci):
            """z-sum matmuls for one exp'd chunk; doubles as PE HAM trickle
            during the prologue (keeps the clock warm between exp batches)."""
            _, nin, _ = SLABS[s]
            cc0, cn = CHUNKS[ci]
            t = pmap[(ti, ci)]
            for wh in (0, 1):
                for c in range(cn):
                    MM(zt[0:nin, wh * WC: (wh + 1) * WC],
                       lhsT=w_sb["ident"][0:nin, 0:nin],
                       rhs=t[0:nin, c, 2 + wh * WC: 2 + (wh + 1) * WC],
                       start=(ci == 0 and c == 0), stop=(ci == 4 and c == cn - 1))

        def emit_z(pmap, s, ti):
            zt = psum.tile([128, 1024], F32, tag="qx", bufs=2)
            for ci in range(5):
                emit_z_chunk(zt, pmap, s, ti, ci)
            return zt

        def emit_recip(s, ti, zt):
            _, nin, _ = SLABS[s]
            r32 = sb.tile([128, W], F32, tag="r32", bufs=2)
            nc.vector.reciprocal_approx_fast(out=r32[0:nin, :],
                                             in_=zt[0:nin, 0:1024])
            r16 = sb.tile([128, W], BF16, tag="r16", bufs=2)
            nc.vector.tensor_copy(out=r16[0:nin, :], in_=r32[0:nin, :])
            return r16

        def emit_norm(pmap, s, ti, r16, cis):
            _, nin, _ = SLABS[s]
            for ci in cis:
                cc0, cn = CHUNKS[ci]
                t = pmap[(ti, ci)]
                for c in range(cn):
                    nc.vector.tensor_mul(out=t[0:nin, c, 2: 2 + W],
                                         in0=t[0:nin, c, 2: 2 + W],
                                         in1=r16[0:nin, :])

        # ---------- conv + extraction for one (slab, channel) ----------
        def emit_conv_cc(pmap, s, cc, mta, mtb):
            _, nin, nout = SLABS[s]
            ci, cl = cc // 4, cc % 4
            va = pmap[(0, ci)][0:nin, cl, :]
            vb = pmap[(1, ci)][0:nin, cl, :]
            # weight-major: each band matrix is loaded once per channel and
            # serves both tensors and both w-halves (4 LDWEIGHTS per channel)
            qx0 = psum.tile([128, 1024], F32, tag="qx", bufs=2)
            qx1 = psum.tile([128, 1024], F32, tag="qx", bufs=2)
            qx = (qx0, qx1)
            for w_name, st, sp, off in (("w_sp", True, False, 3),
                                        ("w_sn", False, True, 1)):
                for wh in (0, 1):
                    b0 = wh * WC
                    for k, v in ((0, va), (1, vb)):
                        MM(qx[wh][:, k * WC: (k + 1) * WC],
                           lhsT=w_sb[w_name][0:nin, :],
                           rhs=v[:, b0 + off: b0 + off + WC],
                           start=st, stop=sp)
            qy0 = psum.tile([128, 1024], F32, tag="qy", bufs=2)
            qy1 = psum.tile([128, 1024], F32, tag="qy", bufs=2)
            qy = (qy0, qy1)
            for w_name, st, sp, offs in (("w_d", True, False, (1, 3)),
                                         ("w_d2", False, True, (2,))):
                for off in offs:
                    for wh in (0, 1):
                        b0 = wh * WC
                        for k, v in ((0, va), (1, vb)):
                            MM(qy[wh][:, k * WC: (k + 1) * WC],
                               lhsT=w_sb[w_name][0:nin, :],
                               rhs=v[:, b0 + off: b0 + off + WC],
                               start=(st and off == 1), stop=sp)
            for wh in (0, 1):
                b0 = wh * WC
                g2 = sb.tile([128, 1024], BF16, tag="g2", bufs=3)
                if (cc * 2 + wh) % 3 < 2:
                    nc.scalar.activation(out=g2[0:nout, :],
                                         in_=qx[wh][0:nout, :], func=SQUARE)
                else:
                    nc.vector._custom_dve(
                        SQSUM, out=g2[0:nout, :], in0=qx[wh][0:nout, :],
                        in1=zeros[0:nout, :], s0=0.0, accum_out=jk[0:nout, :])
                q = sb.tile([128, 1024], BF16, tag="q", bufs=3)
                col = next(qcol)
                nc.vector._custom_dve(
                    SQADD, out=q[0:nout, :], in0=qy[wh][0:nout, :],
                    in1=g2[0:nout, :], s0=0.0,
                    accum_out=acc_sb[0:nout, col: col + 1])
                mdst = mta[0:nout, cc, b0: b0 + WC] if cc < 17 \
                    else mtb[0:nout, cc - 17, b0: b0 + WC]
                nc.gpsimd.tensor_mul(out=mdst, in0=q[0:nout, 0:WC],
                                     in1=q[0:nout, WC: 2 * WC])

        # ---------- prologue: slab0 build + remainder staging ----------
        # z-sum psum slots held across the build; z matmuls trickle in per
        # exp'd chunk so the PE never idles a full HAM window.
        cur_p, nxt_p = {}, {}
        zslot0 = psum.tile([128, 1024], F32, tag="qx", bufs=2)
        zslot1 = psum.tile([128, 1024], F32, tag="qx", bufs=2)
        zslots = {0: zslot0, 1: zslot1}
        for ci in range(5):
            for ti in (0, 1):
                emit_dma_exp(cur_p, 0, ti, ci)
                emit_z_chunk(zslots[ti], cur_p, 0, ti, ci)
                warm_mms(cur_p[(ti, ci)][0:128, 0, 2: 2 + WC], 3)

        # remainder inputs: packed (c, r) partitions, one DMA per tensor
        rps = sb.tile([128, 2, W + 4], BF16, tag="rps", bufs=1)
        nc.vector.memset(rps[0:npk, :, 1: W + 3: W + 1], 0.0)
        rstg = sb.tile([128, 2, W], BF16, tag="rstg", bufs=1)
        for ti in (0, 1):
            nc.sync.dma_start(
                out=rstg[0:npk, ti, :],
                in_=x_dram[ti][REM[0]: REM[0] + blk, :, :]
                .rearrange("h c w -> c h w"))
        nc.scalar.activation(out=rps[0:npk, :, 2: 2 + W],
                             in_=rstg[0:npk, :, :], func=EXP)

        r16s = {}
        for ti in (0, 1):
            r16s[(0, ti)] = emit_recip(0, ti, zslots[ti])
            emit_norm(cur_p, 0, ti, r16s[(0, ti)], (0,))

        # ---------- remainder compute closures (interleaved into slab 1) ----
        rqs = {}

        def rem_build(ti):
            def f():
                zq = psum.tile([128, 1024], F32, tag="qx", bufs=2)
                for wh in (0, 1):
                    MM(zq[0:blk, wh * WC: (wh + 1) * WC],
                       lhsT=w_sb["w_sel"][0:npk, 0:blk],
                       rhs=rps[0:npk, ti, 2 + wh * WC: 2 + (wh + 1) * WC],
                       start=True, stop=True)
                r32 = sb.tile([128, W], F32, tag="r32", bufs=2)
                nc.vector.reciprocal_approx_fast(out=r32[0:blk, :],
                                                 in_=zq[0:blk, 0:1024])
                r16 = sb.tile([128, W], BF16, tag="r16", bufs=2)
                nc.vector.tensor_copy(out=r16[0:blk, :], in_=r32[0:blk, :])
                rrep = psum.tile([128, 1024], F32, tag="qx", bufs=2)
                for wh in (0, 1):
                    MM(rrep[0:npk, wh * WC: (wh + 1) * WC],
                       lhsT=w_sb["w_rep"][0:blk, 0:npk],
                       rhs=r16[0:blk, wh * WC: (wh + 1) * WC],
                       start=True, stop=True)
                nc.vector.tensor_mul(out=rps[0:npk, ti, 2: 2 + W],
                                     in0=rps[0:npk, ti, 2: 2 + W],
                                     in1=rrep[0:npk, 0:1024])
            return f

        def rem_conv(ti):
            def f():
                rq = sb.tile([128, 1024], BF16, tag="rq", bufs=3)
                rqs[ti] = rq
                for wh in (0, 1):
                    b0 = wh * WC
                    qxr = psum.tile([128, 1024], F32, tag="qx", bufs=2)
                    for w_name, st, sp in (("w_rsp", True, False),
                                           ("w_rsn", False, True)):
                        off = 3 if st else 1
                        MM(qxr[:, 0:WC], lhsT=w_sb[w_name][0:npk, :],
                           rhs=rps[0:npk, ti, b0 + off: b0 + off + WC],
                           start=st, stop=sp)
                    qyr = psum.tile([128, 1024], F32, tag="qy", bufs=2)
                    for w_name, off, st, sp in (
                            ("w_rd", 1, True, False), ("w_rd", 3, False, False),
                            ("w_rd2", 2, False, True)):
                        MM(qyr[:, 0:WC], lhsT=w_sb[w_name][0:npk, :],
                           rhs=rps[0:npk, ti, b0 + off: b0 + off + WC],
                           start=st, stop=sp)
                    g2 = sb.tile([128, 1024], BF16, tag="g2", bufs=3)
                    nc.scalar.activation(out=g2[0:npk, 0:WC],
                                         in_=qxr[0:npk, 0:WC], func=SQUARE)
                    col = next(qcol)
                    nc.vector._custom_dve(
                        SQADD, out=rq[0:npk, b0: b0 + WC],
                        in0=qyr[0:npk, 0:WC], in1=g2[0:npk, 0:WC], s0=0.0,
                        accum_out=acc_sb[0:npk, col: col + 1])
            return f

        def rem_tail():
            rm = sb.tile([128, 1024], BF16, tag="rq", bufs=3)
            nc.gpsimd.tensor_mul(out=rm[0:npk, :], in0=rqs[0][0:npk, :],
                                 in1=rqs[1][0:npk, :])
            sc = next(scol)
            nc.scalar.activation(out=rm[0:npk, :], in_=rm[0:npk, :],
                                 func=SQRT, accum_out=acc_sb[0:npk, sc: sc + 1])

        # ---------- main slabs with pipelined next-slab build ----------
        for s in (0, 1):
            mta = sb.tile([128, 17, W], BF16, tag="m", bufs=1)
            mtb = sb.tile([128, 2, W], BF16, tag="mb", bufs=1)
            pops = {}

            def pin(cc, f):
                pops.setdefault(cc, []).append(f)

            def nrm(pmap, ps, ti, ci):
                return lambda: emit_norm(pmap, ps, ti, r16s[(ps, ti)], (ci,))

            # late normalizes of the current slab (chunk ci needed at cc=4*ci)
            cp = dict(cur_p)
            for ci in (1, 2, 3, 4):
                for ti in (0, 1):
                    pin(4 * ci - 4 + ti, nrm(cp, s, ti, ci))
            if s == 0:
                np_ = nxt_p

                def de(ti, ci):
                    return lambda: emit_dma_exp(np_, 1, ti, ci)

                def zr(ti):
                    def f():
                        zt = emit_z(np_, 1, ti)
                        r16s[(1, ti)] = emit_recip(1, ti, zt)
                        emit_norm(np_, 1, ti, r16s[(1, ti)], (0,))
                    return f

                for cc, f in ((2, de(0, 0)), (3, de(1, 0)), (4, de(0, 1)),
                              (5, de(1, 1)), (8, de(0, 2)), (9, de(1, 2)),
                              (12, de(0, 3)), (13, de(1, 3)), (16, de(0, 4)),
                              (17, de(1, 4)), (17, zr(0)), (18, zr(1))):
                    pin(cc, f)
            else:
                pin(2, rem_build(0))
                pin(4, rem_build(1))
                pin(6, rem_conv(0))
                pin(8, rem_conv(1))
                pin(11, rem_tail)

            _, _, nout = SLABS[s]

            def sqrt_part(mti):
                def f():
                    scn = next(scol)
                    nc.scalar.activation(
                        out=mti[0:nout, :, :], in_=mti[0:nout, :, :],
                        func=SQRT, accum_out=acc_sb[0:nout, scn: scn + 1])
                return f

            # bulk of the slab's sqrt runs as soon as channels 0..16 are
            # crossed, leaving only a 2-channel sqrt on the critical tail
            pin(17, sqrt_part(mta))
            for cc in range(C):
                emit_conv_cc(cur_p, s, cc, mta, mtb)
                for f in pops.get(cc, ()):
                    f()
            sqrt_part(mtb)()
            cur_p, nxt_p = nxt_p, {}

        nc.sync.dma_start(out=acc_out[:, :], in_=acc_sb[:, :])
    if not nc.is_finalized():
        nc.finalize()
    return nc


def shard_inputs(student_logits, teacher_logits, c_dim=C, h_dim=H, w_dim=W,
                 ncores=NCORES):
    """Full (B,C,H,W) fp32 -> per-core (rows+2, C, W) bf16 halo shards."""
    b_dim = student_logits.shape[0]
    rows = (b_dim * h_dim) // ncores
    wp = _band_weights()
    in_maps = []
    for k in range(ncores):
        g0 = k * rows
        bi, h0 = g0 // h_dim, g0 % h_dim
        m = {"wpack": wp}
        for name, x in (("xs", student_logits), ("xt", teacher_logits)):
            img = np.asarray(x[bi], np.float32)            # (C, H, W)
            sh = np.zeros((rows + 2, c_dim, w_dim), NP_BF16)
            lo, hi = h0 - 1, h0 + rows + 1
            slo, shi = max(lo, 0), min(hi, h_dim)
            sh[slo - lo: slo - lo + (shi - slo)] = \
                img[:, slo:shi, :].transpose(1, 0, 2).astype(NP_BF16)
            m[name] = sh
        in_maps.append(m)
    return in_maps


_NC_CACHE = {}


def _get_nc():
    if "full" not in _NC_CACHE:
        _NC_CACHE["full"] = build_nc()
    return _NC_CACHE["full"]


def run_on_cores(in_maps, trace=False, **kw):
    nc = _get_nc()
    return bass_utils.run_bass_kernel_spmd(
        nc, in_maps, core_ids=list(range(len(in_maps))), trace=trace, **kw)


def finish(results, n_total=None):
    if n_total is None:
        n_total = B * C * H * W
    tq = 0.0
    tcross = 0.0
    for r in results:
        a = np.asarray(r["acc"], np.float64)
        tq += a[:, :NQ].sum()
        tcross += a[:, NQ: NQ + NS].sum()
    return np.float32((tq - 2.0 * tcross) / n_total)


def kernel(student_logits, teacher_logits):
    in_maps = shard_inputs(np.asarray(student_logits), np.asarray(teacher_logits))
    res = run_on_cores(in_maps)
    return finish(res.results)


# revision 28
# speedup vs baseline: 1.2267x; 1.0182x over previous
"""Trainium2 Bass kernel for nn_BoundaryDistillationLoss.

loss = mean((|grad(softmax(s))| - |grad(softmax(t))|)^2) with depthwise 3x3
Sobel gradients, expanded as [ sum(qs) + sum(qt) - 2*sum(sqrt(qs*qt)) ] / N
with q = gx^2 + gy^2.

Host pre-casts inputs to bf16 and rearranges each core's shard to (h, c, w)
so every DMA is contiguous.  2048 rows data-parallel over 8 cores; per core
two 128-row slabs (126 output rows each) plus a packed 6-row tail that is
interleaved into slab 1.  On-chip: h-rows on partitions, (c, w) on the free
dim, per-4-channel chunk tiles in a ring so slab N+1's DMA/exp/z-sum/
normalize overlaps slab N's conv.  Sobel row-taps are banded 128x128
matmuls emitted weight-major (one LDWEIGHTS per band per channel, both
tensors and both w-halves share it); col-taps are +-1-shifted rhs views of a
zero-padded slab.  gx pairs land in 2-bank psum tiles released by a square
pass (split ~2:1 between ScalarE Square and a fused VectorE SQSUM against a
zeros tile to balance those engines); gy pairs are consumed by a fused
VectorE square-add with free accumulation; GPSIMD multiplies qs*qt; one big
in-place SQRT per slab avoids exp<->sqrt activation-table thrashing.  The
z-sum matmuls double as PE clock-warming trickle during the prologue.
"""

import numpy as np
from contextlib import ExitStack

import concourse.bass as bass
import concourse.bacc as bacc
import concourse.mybir as mybir
import concourse.tile as tile
from concourse import bass_utils
import concourse.dve_ops as dve_ops
from concourse.dve_spec import C0 as _C0, Spec as _Spec, Src0 as _Src0, \
    Src1 as _Src1, lower as _dve_lower, sq as _dve_sq
from concourse.dve_uop import DveOpSpec as _DveOpSpec
from operator import add as _op_add


def _register_custom(name, body, reference):
    for o in dve_ops.OPS:
        if o.name == name:
            return o
    spec = _Spec(body=body, accum=_op_add, accum_init=_C0, reference=reference)
    row = 1 + len(dve_ops.OPS)
    assert row < 0x20
    dve_ops._SUB_OPCODE_FOR_NAME[name] = row
    shas = {}
    for ver in ("v3", "v4"):
        try:
            uops = _dve_lower(spec, ver=ver)
            shas[ver] = _DveOpSpec(name=name, opcode=row, uops=uops,
                                   rd1_en=True).sha(ver)
        except Exception:
            pass
    op = dve_ops.DveOp(name, spec, subdim=False, uops_sha=shas)
    dve_ops.OPS.append(op)
    dve_ops.CUSTOM_DVE_SPECS[name] = spec
    return op


def _ref_sqsum(in0, in1, c0, c1, c2):
    b = (in0.astype(np.float32) ** 2 + in1.astype(np.float32) ** 2).astype(np.float32)
    return b, c0 + b.reshape(b.shape[0], -1).sum(axis=-1, keepdims=True)


def _ref_sqadd(in0, in1, c0, c1, c2):
    b = (in0.astype(np.float32) ** 2 + in1.astype(np.float32)).astype(np.float32)
    return b, c0 + b.reshape(b.shape[0], -1).sum(axis=-1, keepdims=True)


SQSUM = _register_custom("SQSUM_ANT", _dve_sq(_Src0) + _dve_sq(_Src1), _ref_sqsum)
SQADD = _register_custom("SQADD_ANT", _dve_sq(_Src0) + _Src1, _ref_sqadd)

F32 = mybir.dt.float32
BF16 = mybir.dt.bfloat16
NP_BF16 = mybir.dt.np(BF16)

# Problem constants (hardcoded: nn_BoundaryDistillationLoss_87230785781774)
B, C, H, W = 4, 19, 512, 1024
NCORES = 8
ROWS_PER_CORE = (B * H) // NCORES          # 256
HIN = ROWS_PER_CORE + 2                    # 258 (one halo row each side)
SLABS = ((0, 128, 126), (126, 128, 126))   # (in_row_start, n_in, n_out)
REM = (252, 6)                             # packed tail rows 252..257 -> 252..255
CHUNKS = ((0, 4), (4, 4), (8, 4), (12, 4), (16, 3))
WC = 512
PCH_BUFS = 7                               # per-ti chunk-tile ring depth

NQ = 2 * C * 2 + 4                         # SQADD accum cols: (slab,cc,wh) + rem
NS = 2 + 10 + 1                            # slab0 split + slab1 pair + rem cols
NACC = NQ + NS + 1                         # +1 junk col for unused accums
JCOL = NQ + NS


def _shifted_band(a, n, nfull=128):
    """lhsT [nfull, nfull] with lhsT[k, m] = a[m+1, k] (out row m = conv row
    m+1 so consumers start at partition 0); a is [n, n]."""
    t = np.zeros((nfull, nfull), np.float32)
    t[:n, : n - 1] = a.T[:, 1:]
    return t


def _base_bands(n):
    A_s = np.zeros((n, n), np.float32)
    A_d = np.zeros((n, n), np.float32)
    i = np.arange(n)
    A_s[i, i] = 2.0
    A_s[i[:-1], i[:-1] + 1] = 1.0
    A_s[i[1:], i[1:] - 1] = 1.0
    A_d[i[:-1], i[:-1] + 1] = 1.0
    A_d[i[1:], i[1:] - 1] = -1.0
    return A_s, A_d


WNAMES = ("w_sp", "w_sn", "w_d", "w_d2", "ident",
          "w_rsp", "w_rsn", "w_rd", "w_rd2", "w_sel", "w_rep")


def _band_weights(c_dim=C, blk=6):
    A_s, A_d = _base_bands(128)
    out = {
        "w_sp": _shifted_band(A_s, 128),
        "w_sn": _shifted_band(-A_s, 128),
        "w_d": _shifted_band(A_d, 128),
        "w_d2": _shifted_band(2.0 * A_d, 128),
        "ident": np.eye(128, dtype=np.float32),
    }
    a_s, a_d = _base_bands(blk)
    npk = c_dim * blk
    assert npk <= 128
    for name, a in (("w_rsp", a_s), ("w_rsn", -a_s), ("w_rd", a_d),
                    ("w_rd2", 2.0 * a_d)):
        m = np.zeros((128, 128), np.float32)
        sb_ = _shifted_band(a, blk, blk)
        sb_[:, blk - 2:] = 0.0
        for cblk in range(c_dim):
            m[cblk * blk: (cblk + 1) * blk, cblk * blk: (cblk + 1) * blk] = sb_
        out[name] = m
    w_sel = np.zeros((128, 128), np.float32)
    w_rep = np.zeros((128, 128), np.float32)
    for cblk in range(c_dim):
        for i in range(blk):
            w_sel[cblk * blk + i, i] = 1.0
            w_rep[i, cblk * blk + i] = 1.0
    out["w_sel"] = w_sel
    out["w_rep"] = w_rep
    return np.concatenate([out[n].astype(NP_BF16) for n in WNAMES], axis=1)


def build_nc():
    blk = REM[1]
    npk = C * blk

    nc = bacc.Bacc("TRN2", target_bir_lowering=False)
    xs = nc.dram_tensor("xs", [HIN, C, W], BF16, kind="ExternalInput")
    xt = nc.dram_tensor("xt", [HIN, C, W], BF16, kind="ExternalInput")
    wpack = nc.dram_tensor("wpack", [128, 128 * len(WNAMES)], BF16,
                           kind="ExternalInput")
    acc_out = nc.dram_tensor("acc", [128, NACC], F32, kind="ExternalOutput")

    x_dram = (xs, xt)
    EXP = mybir.ActivationFunctionType.Exp
    SQRT = mybir.ActivationFunctionType.Sqrt
    SQUARE = mybir.ActivationFunctionType.Square

    qcol = iter(range(NQ))
    scol = iter(range(NQ, NQ + NS))

    with ExitStack() as ctx:
        tc = ctx.enter_context(tile.TileContext(nc))
        sb = ctx.enter_context(tc.tile_pool(name="sb", bufs=2))
        consts = ctx.enter_context(tc.tile_pool(name="consts", bufs=1))
        psum = ctx.enter_context(tc.tile_pool(name="psum", bufs=1, space="PSUM"))

        wall = consts.tile([128, 128 * len(WNAMES)], BF16, tag="wall")
        nc.sync.dma_start(out=wall, in_=wpack[:, :])
        w_sb = {n: wall[:, i * 128: (i + 1) * 128]
                for i, n in enumerate(WNAMES)}
        acc_sb = consts.tile([128, NACC], F32, tag="acc")
        nc.vector.memset(acc_sb[:, :], 0.0)
        zeros = consts.tile([128, 1024], BF16, tag="zeros")
        nc.vector.memset(zeros[:, :], 0.0)
        jk = acc_sb[:, JCOL: JCOL + 1]

        # tiny dummy EXP: forces the exp activation-table load to happen
        # before the input-DMA stream saturates the DMA path
        dmy = consts.tile([128, 16], BF16, tag="dmy")
        nc.vector.memset(dmy[:, :], 0.0)
        nc.scalar.activation(out=dmy[:, :], in_=dmy[:, :],
                             func=mybir.ActivationFunctionType.Exp)

        MM = nc.tensor.matmul

        def warm_mms(rhs, n):
            # dummy matmuls that depend on freshly-exp'd data: paced through
            # the prologue they keep the PE HAM activity window non-idle
            wp = psum.tile([128, 1024], F32, tag="qy", bufs=2)
            for _ in range(n):
                MM(wp[:, 0:WC], lhsT=w_sb["ident"][:, :], rhs=rhs,
                   start=True, stop=True)

        # ---------- build-phase helpers (per slab, tensor) ----------
        def emit_dma_exp(pmap, s, ti, ci):
            r0, nin, _ = SLABS[s]
            cc0, cn = CHUNKS[ci]
            t = sb.tile([128, 4, W + 4], BF16, tag=f"pch{ti}", bufs=PCH_BUFS)
            # zero the conv border cols (1 and W+2); cols 0 / W+3 unused
            nc.vector.memset(t[0:nin, 0:cn, 1: W + 3: W + 1], 0.0)
            nc.sync.dma_start(
                out=t[0:nin, 0:cn, 2: 2 + W],
                in_=x_dram[ti][r0: r0 + nin, cc0: cc0 + cn, :])
            nc.scalar.activation(out=t[0:nin, 0:cn, 2: 2 + W],
                                 in_=t[0:nin, 0:cn, 2: 2 + W], func=EXP)
            pmap[(ti, ci)] = t

        def emit_z_chunk(zt, pmap, s, ti, ci):
            """z-sum matmuls for one exp'd chunk; doubles as PE HAM trickle
            during the prologue (keeps the clock warm between exp batches)."""
            _, nin, _ = SLABS[s]
            cc0, cn = CHUNKS[ci]
            t = pmap[(ti, ci)]
            for wh in (0, 1):
                for c in range(cn):
                    MM(zt[0:nin, wh * WC: (wh + 1) * WC],
                       lhsT=w_sb["ident"][0:nin, 0:nin],
                       rhs=t[0:nin, c, 2 + wh * WC: 2 + (wh + 1) * WC],
                       start=(ci == 0 and c == 0), stop=(ci == 4 and c == cn - 1))

        def emit_z(pmap, s, ti):
            zt = psum.tile([128, 1024], F32, tag="qx", bufs=2)
            for ci in range(5):
                emit_z_chunk(zt, pmap, s, ti, ci)
            return zt

        def emit_recip(s, ti, zt):
            _, nin, _ = SLABS[s]
            r32 = sb.tile([128, W], F32, tag="r32", bufs=2)
            nc.vector.reciprocal_approx_fast(out=r32[0:nin, :],
                                             in_=zt[0:nin, 0:1024])
            r16 = sb.tile([128, W], BF16, tag="r16", bufs=2)
            nc.vector.tensor_copy(out=r16[0:nin, :], in_=r32[0:nin, :])
            return r16

        def emit_norm(pmap, s, ti, r16, cis):
            _, nin, _ = SLABS[s]
            for ci in cis:
                cc0, cn = CHUNKS[ci]
                t = pmap[(ti, ci)]
                for c in range(cn):
                    nc.vector.tensor_mul(out=t[0:nin, c, 2: 2 + W],
                                         in0=t[0:nin, c, 2: 2 + W],
                                         in1=r16[0:nin, :])

        # ---------- conv + extraction for one (slab, channel) ----------
        def emit_conv_cc(pmap, s, cc, get_m):
            _, nin, nout = SLABS[s]
            ci, cl = cc // 4, cc % 4
            va = pmap[(0, ci)][0:nin, cl, :]
            vb = pmap[(1, ci)][0:nin, cl, :]
            # weight-major: each band matrix is loaded once per channel and
            # serves both tensors and both w-halves (4 LDWEIGHTS per channel)
            qx0 = psum.tile([128, 1024], F32, tag="qx", bufs=2)
            qx1 = psum.tile([128, 1024], F32, tag="qx", bufs=2)
            qx = (qx0, qx1)
            for w_name, st, sp, off in (("w_sp", True, False, 3),
                                        ("w_sn", False, True, 1)):
                for wh in (0, 1):
                    b0 = wh * WC
                    for k, v in ((0, va), (1, vb)):
                        MM(qx[wh][:, k * WC: (k + 1) * WC],
                           lhsT=w_sb[w_name][0:nin, :],
                           rhs=v[:, b0 + off: b0 + off + WC],
                           start=st, stop=sp)
            qy0 = psum.tile([128, 1024], F32, tag="qy", bufs=2)
            qy1 = psum.tile([128, 1024], F32, tag="qy", bufs=2)
            qy = (qy0, qy1)
            for w_name, st, sp, offs in (("w_d", True, False, (1, 3)),
                                         ("w_d2", False, True, (2,))):
                for off in offs:
                    for wh in (0, 1):
                        b0 = wh * WC
                        for k, v in ((0, va), (1, vb)):
                            MM(qy[wh][:, k * WC: (k + 1) * WC],
                               lhsT=w_sb[w_name][0:nin, :],
                               rhs=v[:, b0 + off: b0 + off + WC],
                               start=(st and off == 1), stop=sp)
            for wh in (0, 1):
                b0 = wh * WC
                g2 = sb.tile([128, 1024], BF16, tag="g2", bufs=3)
                if (cc * 2 + wh) % 3 < 2:
                    nc.scalar.activation(out=g2[0:nout, :],
                                         in_=qx[wh][0:nout, :], func=SQUARE)
                else:
                    nc.vector._custom_dve(
                        SQSUM, out=g2[0:nout, :], in0=qx[wh][0:nout, :],
                        in1=zeros[0:nout, :], s0=0.0, accum_out=jk[0:nout, :])
                q = sb.tile([128, 1024], BF16, tag="q", bufs=3)
                col = next(qcol)
                nc.vector._custom_dve(
                    SQADD, out=q[0:nout, :], in0=qy[wh][0:nout, :],
                    in1=g2[0:nout, :], s0=0.0,
                    accum_out=acc_sb[0:nout, col: col + 1])
                nc.gpsimd.tensor_mul(out=get_m(cc, b0), in0=q[0:nout, 0:WC],
                                     in1=q[0:nout, WC: 2 * WC])

        # ---------- prologue: slab0 build + remainder staging ----------
        # z-sum psum slots held across the build; z matmuls trickle in per
        # exp'd chunk so the PE never idles a full HAM window.
        cur_p, nxt_p = {}, {}
        zslot0 = psum.tile([128, 1024], F32, tag="qx", bufs=2)
        zslot1 = psum.tile([128, 1024], F32, tag="qx", bufs=2)
        zslots = {0: zslot0, 1: zslot1}
        for ci in range(5):
            for ti in (0, 1):
                emit_dma_exp(cur_p, 0, ti, ci)
                emit_z_chunk(zslots[ti], cur_p, 0, ti, ci)
                warm_mms(cur_p[(ti, ci)][0:128, 0, 2: 2 + WC], 3)

        # remainder inputs: packed (c, r) partitions, one DMA per tensor
        rps = sb.tile([128, 2, W + 4], BF16, tag="rps", bufs=1)
        nc.vector.memset(rps[0:npk, :, 1: W + 3: W + 1], 0.0)
        rstg = sb.tile([128, 2, W], BF16, tag="rstg", bufs=1)
        for ti in (0, 1):
            nc.sync.dma_start(
                out=rstg[0:npk, ti, :],
                in_=x_dram[ti][REM[0]: REM[0] + blk, :, :]
                .rearrange("h c w -> c h w"))
        nc.scalar.activation(out=rps[0:npk, :, 2: 2 + W],
                             in_=rstg[0:npk, :, :], func=EXP)

        r16s = {}
        for ti in (0, 1):
            r16s[(0, ti)] = emit_recip(0, ti, zslots[ti])
            emit_norm(cur_p, 0, ti, r16s[(0, ti)], (0,))

        # ---------- remainder compute closures (interleaved into slab 1) ----
        rqs = {}

        def rem_build(ti):
            def f():
                zq = psum.tile([128, 1024], F32, tag="qx", bufs=2)
                for wh in (0, 1):
                    MM(zq[0:blk, wh * WC: (wh + 1) * WC],
                       lhsT=w_sb["w_sel"][0:npk, 0:blk],
                       rhs=rps[0:npk, ti, 2 + wh * WC: 2 + (wh + 1) * WC],
                       start=True, stop=True)
                r32 = sb.tile([128, W], F32, tag="r32", bufs=2)
                nc.vector.reciprocal_approx_fast(out=r32[0:blk, :],
                                                 in_=zq[0:blk, 0:1024])
                r16 = sb.tile([128, W], BF16, tag="r16", bufs=2)
                nc.vector.tensor_copy(out=r16[0:blk, :], in_=r32[0:blk, :])
                rrep = psum.tile([128, 1024], F32, tag="qx", bufs=2)
                for wh in (0, 1):
                    MM(rrep[0:npk, wh * WC: (wh + 1) * WC],
                       lhsT=w_sb["w_rep"][0:blk, 0:npk],
                       rhs=r16[0:blk, wh * WC: (wh + 1) * WC],
                       start=True, stop=True)
                nc.vector.tensor_mul(out=rps[0:npk, ti, 2: 2 + W],
                                     in0=rps[0:npk, ti, 2: 2 + W],
                                     in1=rrep[0:npk, 0:1024])
            return f

        def rem_conv(ti):
            def f():
                rq = sb.tile([128, 1024], BF16, tag="rq", bufs=3)
                rqs[ti] = rq
                for wh in (0, 1):
                    b0 = wh * WC
                    qxr = psum.tile([128, 1024], F32, tag="qx", bufs=2)
                    for w_name, st, sp in (("w_rsp", True, False),
                                           ("w_rsn", False, True)):
                        off = 3 if st else 1
                        MM(qxr[:, 0:WC], lhsT=w_sb[w_name][0:npk, :],
                           rhs=rps[0:npk, ti, b0 + off: b0 + off + WC],
                           start=st, stop=sp)
                    qyr = psum.tile([128, 1024], F32, tag="qy", bufs=2)
                    for w_name, off, st, sp in (
                            ("w_rd", 1, True, False), ("w_rd", 3, False, False),
                            ("w_rd2", 2, False, True)):
                        MM(qyr[:, 0:WC], lhsT=w_sb[w_name][0:npk, :],
                           rhs=rps[0:npk, ti, b0 + off: b0 + off + WC],
                           start=st, stop=sp)
                    g2 = sb.tile([128, 1024], BF16, tag="g2", bufs=3)
                    nc.scalar.activation(out=g2[0:npk, 0:WC],
                                         in_=qxr[0:npk, 0:WC], func=SQUARE)
                    col = next(qcol)
                    nc.vector._custom_dve(
                        SQADD, out=rq[0:npk, b0: b0 + WC],
                        in0=qyr[0:npk, 0:WC], in1=g2[0:npk, 0:WC], s0=0.0,
                        accum_out=acc_sb[0:npk, col: col + 1])
            return f

        def rem_tail():
            rm = sb.tile([128, 1024], BF16, tag="rq", bufs=3)
            nc.gpsimd.tensor_mul(out=rm[0:npk, :], in0=rqs[0][0:npk, :],
                                 in1=rqs[1][0:npk, :])
            sc = next(scol)
            nc.scalar.activation(out=rm[0:npk, :], in_=rm[0:npk, :],
                                 func=SQRT, accum_out=acc_sb[0:npk, sc: sc + 1])

        # ---------- main slabs with pipelined next-slab build ----------
        for s in (0, 1):
            pops = {}

            def pin(cc, f):
                pops.setdefault(cc, []).append(f)

            def nrm(pmap, ps, ti, ci):
                return lambda: emit_norm(pmap, ps, ti, r16s[(ps, ti)], (ci,))

            # late normalizes of the current slab (chunk ci needed at cc=4*ci)
            cp = dict(cur_p)
            for ci in (1, 2, 3, 4):
                for ti in (0, 1):
                    pin(4 * ci - 4 + ti, nrm(cp, s, ti, ci))
            if s == 0:
                np_ = nxt_p

                def de(ti, ci):
                    return lambda: emit_dma_exp(np_, 1, ti, ci)

                def zr(ti):
                    def f():
                        zt = emit_z(np_, 1, ti)
                        r16s[(1, ti)] = emit_recip(1, ti, zt)
                        emit_norm(np_, 1, ti, r16s[(1, ti)], (0,))
                    return f

                for cc, f in ((2, de(0, 0)), (3, de(1, 0)), (4, de(0, 1)),
                              (5, de(1, 1)), (8, de(0, 2)), (9, de(1, 2)),
                              (12, de(0, 3)), (13, de(1, 3)), (15, de(0, 4)),
                              (15, de(1, 4)), (16, zr(0)), (17, zr(1))):
                    pin(cc, f)
            else:
                pin(2, rem_build(0))
                pin(4, rem_build(1))
                pin(6, rem_conv(0))
                pin(8, rem_conv(1))
                pin(11, rem_tail)

            _, _, nout = SLABS[s]

            if s == 0:
                # slab0 interleaves next-slab EXPs on ScalarE, so per-channel
                # sqrts would thrash activation tables: accumulate into two
                # big tiles and sqrt them late (they overlap slab1's work).
                mta = sb.tile([128, 17, W], BF16, tag="m", bufs=1)
                mtb = sb.tile([128, 2, W], BF16, tag="mb", bufs=1)

                def get_m(cc, b0):
                    return mta[0:nout, cc, b0: b0 + WC] if cc < 17 \
                        else mtb[0:nout, cc - 17, b0: b0 + WC]

                def sqrt_part(mti):
                    def f():
                        scn = next(scol)
                        nc.scalar.activation(
                            out=mti[0:nout, :, :], in_=mti[0:nout, :, :],
                            func=SQRT, accum_out=acc_sb[0:nout, scn: scn + 1])
                    return f

                pin(17, sqrt_part(mta))
                for cc in range(C):
                    emit_conv_cc(cur_p, s, cc, get_m)
                    for f in pops.get(cc, ()):
                        f()
                sqrt_part(mtb)()
            else:
                # slab1 has no EXPs left: sqrt each channel pair as soon as
                # its cross-products land, keeping the tail to ~one pair
                mp_box = [None]

                def get_m(cc, b0):
                    return mp_box[0][0:nout, cc % 2, b0: b0 + WC]

                for cc in range(C):
                    if cc % 2 == 0:
                        mp_t = sb.tile([128, 2, W], BF16, tag="mp", bufs=3)
                        mp_box[0] = mp_t
                    emit_conv_cc(cur_p, s, cc, get_m)
                    if cc % 2 == 1 or cc == C - 1:
                        npair = cc % 2 + 1
                        scn = next(scol)
                        mp = mp_box[0]
                        nc.scalar.activation(
                            out=mp[0:nout, 0:npair, :],
                            in_=mp[0:nout, 0:npair, :], func=SQRT,
                            accum_out=acc_sb[0:nout, scn: scn + 1])
                    for f in pops.get(cc, ()):
                        f()
            cur_p, nxt_p = nxt_p, {}

        nc.sync.dma_start(out=acc_out[:, :], in_=acc_sb[:, :])
    if not nc.is_finalized():
        nc.finalize()
    return nc


def shard_inputs(student_logits, teacher_logits, c_dim=C, h_dim=H, w_dim=W,
                 ncores=NCORES):
    """Full (B,C,H,W) fp32 -> per-core (rows+2, C, W) bf16 halo shards."""
    b_dim = student_logits.shape[0]
    rows = (b_dim * h_dim) // ncores
    wp = _band_weights()
    in_maps = []
    for k in range(ncores):
        g0 = k * rows
        bi, h0 = g0 // h_dim, g0 % h_dim
        m = {"wpack": wp}
        for name, x in (("xs", student_logits), ("xt", teacher_logits)):
            img = np.asarray(x[bi], np.float32)            # (C, H, W)
            sh = np.zeros((rows + 2, c_dim, w_dim), NP_BF16)
            lo, hi = h0 - 1, h0 + rows + 1
            slo, shi = max(lo, 0), min(hi, h_dim)
            sh[slo - lo: slo - lo + (shi - slo)] = \
                img[:, slo:shi, :].transpose(1, 0, 2).astype(NP_BF16)
            m[name] = sh
        in_maps.append(m)
    return in_maps


_NC_CACHE = {}


def _get_nc():
    if "full" not in _NC_CACHE:
        _NC_CACHE["full"] = build_nc()
    return _NC_CACHE["full"]


def run_on_cores(in_maps, trace=False, **kw):
    nc = _get_nc()
    return bass_utils.run_bass_kernel_spmd(
        nc, in_maps, core_ids=list(range(len(in_maps))), trace=trace, **kw)


def finish(results, n_total=None):
    if n_total is None:
        n_total = B * C * H * W
    tq = 0.0
    tcross = 0.0
    for r in results:
        a = np.asarray(r["acc"], np.float64)
        tq += a[:, :NQ].sum()
        tcross += a[:, NQ: NQ + NS].sum()
    return np.float32((tq - 2.0 * tcross) / n_total)


def kernel(student_logits, teacher_logits):
    in_maps = shard_inputs(np.asarray(student_logits), np.asarray(teacher_logits))
    res = run_on_cores(in_maps)
    return finish(res.results)
